# revision 2
# baseline (speedup 1.0000x reference)
"""Trainium2 Bass kernel v2: graph-per-core cross-attention.

Math (reference):
    f       = Wf @ f_pre_in.T + bf                      (H, M)
    b_feat  = Wb @ b_pre_in[g] + bb                     per graph (H, N)
    w_euc   = softmax((b_feat[g,:32].T @ f[:32]) / 8)   per node over N
    w_geo   = softmax((b_feat[g,32:].T @ f[32:]) / 8)
    out     = [bv_feat[g] @ w_euc, bv_feat[g] @ w_geo] @ Wo.T + bo

v2 strategy vs the chunked baseline:
  * one graph per core (B == n_cores == 8): boundary tensors are loaded
    and transformed exactly once per core, node padding is max_graph
    rounded up (544 for the 4096/8 multinomial) instead of 3x256 chunk
    slots.
  * Wo is folded into the bv path on the host (weight-weight constant
    folding): bvo_h = (Wo_h @ Wbv) @ bv + (Wo_h @ bbv + bo/2), so the
    final projection matmul, its PSUM round-trip and the concat vanish.
    A ones channel makes column 64 of the apply output the softmax
    denominator.
  * apply runs "orientation A": exp-score tiles are the PE stationary
    (ldweights are free), bvo (128n x 66) is the moving operand, so the
    output lands as (node, feature) with the denominator as a
    per-partition scalar -- normalization is two cheap DVE ops per
    128-node chunk and the output DMAs out untransposed.
  * all heavy matmuls run in bf16 (1 cycle/row at any free size; halves
    DMA and SBUF traffic).  Accumulation stays fp32 in PSUM.
  * ACT runs exp only (no table thrashing); biases ride the mandatory
    PSUM->SBUF copies on DVE.
"""

import sys

for _p in ("/opt/trn_rl_repo", "/root/.axon_site/_ro/trn_rl_repo"):
    if _p not in sys.path:
        sys.path.append(_p)

import numpy as np

import bass_rust

import concourse.bass as bass
import concourse.mybir as mybir
from concourse.bass_utils import run_bass_kernel_spmd
from concourse.tile import TileContext
from concourse.vector_clock import ScopedClock, VectorClock

F32 = mybir.dt.float32
BF16 = mybir.dt.bfloat16
AL = mybir.AluOpType

# walrus build here rejects >1 semaphore wait per instruction; split the
# excess onto nops (same workaround as the baseline kernel).
_MAX_WAITS = 1

M, B, N, FD, BD, BVD, H = 4096, 8, 512, 128, 128, 6, 64
H2 = H // 2
N_CORES = 8
MW_DEFAULT = 544


class _ChunkedDrainTileContext(TileContext):
    _nop_uid = 0

    def _add_instruction(self, inst):
        si = inst.sync_info
        if (
            si is not None
            and si.on_wait
            and len(si.on_wait) > _MAX_WAITS
            and inst.engine != mybir.EngineType.Unassigned
        ):
            waits = list(si.on_wait)
            excess, keep = waits[:-_MAX_WAITS], waits[-_MAX_WAITS:]
            for i in range(0, len(excess), _MAX_WAITS):
                _ChunkedDrainTileContext._nop_uid += 1
                nop = mybir.InstNoOp(
                    name=f"splitw{_ChunkedDrainTileContext._nop_uid}", ins=[], outs=[]
                )
                nop.engine = inst.engine
                nop.sync_info = bass_rust.SyncInfo(
                    on_wait=excess[i : i + _MAX_WAITS], on_update=[]
                )
                super()._add_instruction(nop)
            inst.sync_info = bass_rust.SyncInfo(on_wait=keep, on_update=si.on_update)
        super()._add_instruction(inst)

    def _drain_and_barrier(self, tick_clock, wait_clock):
        nc = self.nc
        g = tick_clock.global_clock
        nprocs = len(g)
        for i in range(nprocs):
            if g[i] > 0:
                vc = VectorClock([0] * nprocs)
                vc.require_at_least(i, g[i])
                nop_inst = nc.sync.nop(nofuse=True, hint=f"drain_wait_p{i}")
                wait_clock.add_sem_waits(nop_inst.ins, ScopedClock({None: vc}))
        nc.sync.drain()
        nc.all_engine_barrier()
        assert self.sems is not None
        popped = nc._tile_sem_poison_stack.pop()
        assert popped is self._sem_poison
        nc.clear_and_free_semaphores(list(self.sems.allocated().values()))
        nc.all_engine_barrier()


def _mslots(mw):
    """Slot list [(offset, length)] with length<=256 (PSUM bank size)."""
    slots = []
    off = 0
    while off < mw:
        slots.append((off, min(256, mw - off)))
        off += 256
    return slots


def _mchunks(mw):
    """Apply chunks [(offset, length)] with length<=128 (stationary)."""
    chunks = []
    off = 0
    while off < mw:
        chunks.append((off, min(128, mw - off)))
        off += 128
    return chunks


def build_program(reps=1, mw=MW_DEFAULT):
    nc = bass.Bass()

    d_f = nc.declare_dram_parameter("ft", [FD, mw], BF16, isOutput=False)
    d_b = nc.declare_dram_parameter("bp", [BD, N], BF16, isOutput=False)
    d_bv = nc.declare_dram_parameter("bvp", [8, N], BF16, isOutput=False)
    # wbf: WfT 0:64 | WbT 64:128 | wbvo_e 128:194 (rows 0:8) | wbvo_g 194:260
    d_wb = nc.declare_dram_parameter("wbf", [128, 260], BF16, isOutput=False)
    # wsc: col0 bf, col1 bb
    d_ws = nc.declare_dram_parameter("wsc", [64, 2], F32, isOutput=False)
    d_out = nc.declare_dram_parameter("outm", [mw, H], F32, isOutput=True)

    slots = _mslots(mw)
    chunks = _mchunks(mw)

    with _ChunkedDrainTileContext(nc) as tc, nc.allow_low_precision(
        reason="bf16 compute of fp32 data"
    ):
        with (
            tc.tile_pool(name="const", bufs=1) as cp,
            tc.tile_pool(name="io", bufs=2) as iop,
            tc.tile_pool(name="wk", bufs=2) as wkp,
            tc.tile_pool(name="te", bufs=3) as tep,
            tc.tile_pool(name="ps_f", bufs=1, space="PSUM") as psp_f,
            tc.tile_pool(name="ps_v", bufs=1, space="PSUM") as psp_v,
            tc.tile_pool(name="ps_s", bufs=2, space="PSUM") as psp_s,
            tc.tile_pool(name="ps_a", bufs=2, space="PSUM") as psp_a,
        ):
            t_w = cp.tile([128, 260], BF16, tag="wbf")
            nc.sync.dma_start(t_w[:], d_wb[:])
            t_wft = t_w[:, 0:64]
            t_wbt = t_w[:, 64:128]
            t_wbv = {"e": t_w[0:8, 128:194], "g": t_w[0:8, 194:260]}
            t_ws = cp.tile([64, 2], F32, tag="wsc")
            nc.scalar.dma_start(t_ws[:], d_ws[:])
            t_bfc = t_ws[:, 0:1]
            t_bbc = t_ws[:, 1:2]
            t_bvp = cp.tile([8, N], BF16, tag="bvp")
            nc.gpsimd.dma_start(t_bvp[:], d_bv[:])

            for rep in range(reps):
                # ---- loads ----
                t_ft = iop.tile([FD, mw], BF16, tag="f")
                nc.sync.dma_start(t_ft[:], d_f[:])
                t_bt = iop.tile([BD, N], BF16, tag="b")
                nc.scalar.dma_start(t_bt[:], d_b[:])

                # ---- node features f = Wf @ fT + bf   (64, mw) bf16 ----
                t_fsb = wkp.tile([H, mw], BF16, tag="fsb")
                for so, sl in slots:
                    ps = psp_f.tile([H, N], F32, tag="feat")
                    nc.tensor.matmul(
                        ps[:, 0:sl], t_wft, t_ft[:, so : so + sl],
                        start=True, stop=True,
                    )
                    nc.vector.tensor_scalar_add(
                        t_fsb[:, so : so + sl], ps[:, 0:sl], t_bfc
                    )

                # ---- boundary features b = Wb @ bp + bb   (64, 512) ----
                ps_b = psp_f.tile([H, N], F32, tag="feat")
                nc.tensor.matmul(ps_b[:], t_wbt, t_bt[:], start=True, stop=True)
                t_bsb = wkp.tile([H, N], BF16, tag="bsb")
                nc.vector.tensor_scalar_add(t_bsb[:], ps_b[:], t_bbc)

                # ---- bvo_h[n, j] = (Wo_h@Wbv @ bv)[j,n] + fold, col64=1 ----
                t_bvo = {}
                for hx in ("e", "g"):
                    ps_v = psp_v.tile([128, 264], F32, tag="bvo")
                    for k in range(4):
                        nc.tensor.matmul(
                            ps_v[:, 66 * k : 66 * k + 66],
                            t_bvp[:, 128 * k : 128 * k + 128],
                            t_wbv[hx],
                            start=True, stop=True,
                        )
                    tb = wkp.tile([128, 264], BF16, tag=f"bvo{hx}")
                    nc.vector.tensor_copy(tb[:], ps_v[:])
                    t_bvo[hx] = tb

                # ---- scores -> exp -> apply -> normalize, per slot ----
                te = {}
                ps_apply = {}

                def emit_scores(si_):
                    so, sl = slots[si_]
                    for hi, hx in ((0, "e"), (1, "g")):
                        h0 = hi * H2
                        ps_s = psp_s.tile([128, 1024], F32, tag="s")
                        for k in range(4):
                            nc.tensor.matmul(
                                ps_s[:, sl * k : sl * k + sl],
                                t_bsb[h0 : h0 + H2, 128 * k : 128 * k + 128],
                                t_fsb[h0 : h0 + H2, so : so + sl],
                                start=True, stop=True,
                            )
                        t_e = tep.tile([128, 1024], BF16, tag="te")
                        nc.scalar.activation(
                            t_e[:, 0 : 4 * sl],
                            ps_s[:, 0 : 4 * sl],
                            mybir.ActivationFunctionType.Exp,
                            scale=0.125,
                        )
                        te[(si_, hx)] = t_e

                def emit_apply(si_):
                    so, sl = slots[si_]
                    ps_a = psp_a.tile([128, 264], F32, tag="ap")
                    segs = []
                    for j in range(0, sl, 128):
                        ml = min(128, sl - j)
                        for hx in ("e", "g"):
                            seg = 66 * len(segs)
                            for k in range(4):
                                nc.tensor.matmul(
                                    ps_a[0:ml, seg : seg + 66],
                                    te[(si_, hx)][:, sl * k + j : sl * k + j + ml],
                                    t_bvo[hx][:, 66 * k : 66 * k + 66],
                                    start=(k == 0), stop=(k == 3),
                                )
                            segs.append((j, ml, hx, seg))
                    ps_apply[si_] = (ps_a, segs)

                def emit_norm(si_):
                    so, sl = slots[si_]
                    ps_a, segs = ps_apply[si_]
                    for i in range(0, len(segs), 2):
                        j, ml, _, seg_e = segs[i]
                        _, _, _, seg_g = segs[i + 1]
                        t_r = wkp.tile([128, 2], F32, tag="recip")
                        nc.vector.reciprocal(
                            t_r[0:ml, 0:1], ps_a[0:ml, seg_e + 64 : seg_e + 65]
                        )
                        nc.vector.reciprocal(
                            t_r[0:ml, 1:2], ps_a[0:ml, seg_g + 64 : seg_g + 65]
                        )
                        t_t = wkp.tile([128, H], F32, tag="tmp")
                        nc.vector.tensor_scalar_mul(
                            t_t[0:ml, :], ps_a[0:ml, seg_e : seg_e + 64],
                            t_r[0:ml, 0:1],
                        )
                        t_f_ = iop.tile([128, H], F32, tag="fin")
                        nc.vector.scalar_tensor_tensor(
                            t_f_[0:ml, :],
                            ps_a[0:ml, seg_g : seg_g + 64],
                            t_r[0:ml, 1:2],
                            t_t[0:ml, :],
                            AL.mult,
                            AL.add,
                        )
                        nc.sync.dma_start(
                            d_out[so + j : so + j + ml, :], t_f_[0:ml, :]
                        )

                # interleave so PE stays busy while ACT/DVE trail
                emit_scores(0)
                if len(slots) > 1:
                    emit_scores(1)
                emit_apply(0)
                for si_ in range(2, len(slots)):
                    emit_scores(si_)
                emit_norm(0)
                for si_ in range(1, len(slots)):
                    emit_apply(si_)
                    emit_norm(si_)

    return nc


def plan(batch):
    batch = np.asarray(batch).astype(np.int64)
    bounds = np.searchsorted(batch, np.arange(B + 1))
    sizes = np.diff(bounds)
    mw = max(288, int(-(-sizes.max() // 32) * 32))
    return bounds, mw


def stage_inputs(inputs, bounds, mw):
    f_pre_in = np.asarray(inputs["f_pre_in"], dtype=np.float32)
    b_pre_in = np.asarray(inputs["b_pre_in"], dtype=np.float32)
    bv_in = np.asarray(inputs["bv_in"], dtype=np.float32)
    Wf = np.asarray(inputs["Wf"], dtype=np.float32)
    bf = np.asarray(inputs["bf"], dtype=np.float32)
    Wb = np.asarray(inputs["Wb"], dtype=np.float32)
    bb = np.asarray(inputs["bb"], dtype=np.float32)
    Wbv = np.asarray(inputs["Wbv"], dtype=np.float32)
    bbv = np.asarray(inputs["bbv"], dtype=np.float32)
    Wo = np.asarray(inputs["Wo"], dtype=np.float32)
    bo = np.asarray(inputs["bo"], dtype=np.float32)

    wbf = np.zeros((128, 260), np.float32)
    wbf[:, 0:64] = Wf.T
    wbf[:, 64:128] = Wb.T
    for ci, h0 in ((0, 0), (1, H)):
        wo_h = Wo[:, h0 : h0 + H]                     # (64, 64)
        wv = wo_h @ Wbv                               # (64, 6)
        cv = wo_h @ bbv + 0.5 * bo                    # (64,)
        blk = np.zeros((8, 66), np.float32)
        blk[0:6, 0:64] = wv.T
        blk[6, 0:64] = cv
        blk[6, 64] = 1.0
        wbf[0:8, 128 + 66 * ci : 194 + 66 * ci] = blk
    wsc = np.stack([bf, bb], axis=1)

    in_maps = []
    for g in range(N_CORES):
        s, e = int(bounds[g]), int(bounds[g + 1])
        ft = np.zeros((FD, mw), np.float32)
        ft[:, 0 : e - s] = f_pre_in[s:e].T
        bvp = np.zeros((8, N), np.float32)
        bvp[0:6] = bv_in[g]
        bvp[6] = 1.0
        in_maps.append(
            {
                "ft": ft.astype(np.float32),  # cast below
                "bp": b_pre_in[g],
                "bvp": bvp,
                "wbf": wbf,
                "wsc": wsc,
            }
        )
    # bf16 conversion via jax-free numpy view trick: round-to-nearest-even
    import ml_dtypes

    for m_ in in_maps:
        for k in ("ft", "bp", "bvp", "wbf"):
            m_[k] = np.asarray(m_[k], dtype=ml_dtypes.bfloat16)
    return in_maps


def unstage_output(results, bounds, mw):
    out = np.zeros((M, H), np.float32)
    for g in range(N_CORES):
        s, e = int(bounds[g]), int(bounds[g + 1])
        out[s:e] = results[g]["outm"][0 : e - s]
    return out


_NC_CACHE = {}


def kernel(**inputs):
    assert np.asarray(inputs["f_pre_in"]).shape == (M, FD)
    bounds, mw = plan(inputs["f_pre_batch"])
    in_maps = stage_inputs(inputs, bounds, mw)
    if mw not in _NC_CACHE:
        _NC_CACHE[mw] = build_program(mw=mw)
    nc = _NC_CACHE[mw]
    res = run_bass_kernel_spmd(nc, in_maps, core_ids=list(range(N_CORES)))
    return unstage_output(res.results, bounds, mw)


if __name__ == "__main__":
    rng = np.random.default_rng(0)
    demo = {
        "f_pre_in": rng.standard_normal((M, FD), dtype=np.float32),
        "f_pre_batch": np.sort(rng.integers(0, B, size=M)),
        "b_pre_in": rng.standard_normal((B, BD, N), dtype=np.float32),
        "bv_in": rng.standard_normal((B, BVD, N), dtype=np.float32),
        "Wf": rng.standard_normal((H, FD), dtype=np.float32) * 0.05,
        "bf": rng.standard_normal(H, dtype=np.float32) * 0.05,
        "Wb": rng.standard_normal((H, BD), dtype=np.float32) * 0.05,
        "bb": rng.standard_normal(H, dtype=np.float32) * 0.05,
        "Wbv": rng.standard_normal((H, BVD), dtype=np.float32) * 0.05,
        "bbv": rng.standard_normal(H, dtype=np.float32) * 0.05,
        "Wo": rng.standard_normal((H, 2 * H), dtype=np.float32) * 0.05,
        "bo": rng.standard_normal(H, dtype=np.float32) * 0.05,
    }
    out = kernel(**demo)
    print("kernel output", out.shape, out.dtype, float(np.abs(out).mean()))


# revision 3
# speedup vs baseline: 1.8494x; 1.8494x over previous
"""Trainium2 Bass kernel v2: graph-per-core cross-attention.

Math (reference):
    f       = Wf @ f_pre_in.T + bf                      (H, M)
    b_feat  = Wb @ b_pre_in[g] + bb                     per graph (H, N)
    w_euc   = softmax((b_feat[g,:32].T @ f[:32]) / 8)   per node over N
    w_geo   = softmax((b_feat[g,32:].T @ f[32:]) / 8)
    out     = [bv_feat[g] @ w_euc, bv_feat[g] @ w_geo] @ Wo.T + bo

v2 strategy vs the chunked baseline:
  * one graph per core (B == n_cores == 8): boundary tensors are loaded
    and transformed exactly once per core, node padding is max_graph
    rounded up (544 for the 4096/8 multinomial) instead of 3x256 chunk
    slots.
  * Wo is folded into the bv path on the host (weight-weight constant
    folding): bvo_h = (Wo_h @ Wbv) @ bv + (Wo_h @ bbv + bo/2), so the
    final projection matmul, its PSUM round-trip and the concat vanish.
    A ones channel makes column 64 of the apply output the softmax
    denominator.
  * apply runs "orientation A": exp-score tiles are the PE stationary
    (ldweights are free), bvo (128n x 66) is the moving operand, so the
    output lands as (node, feature) with the denominator as a
    per-partition scalar -- normalization is two cheap DVE ops per
    128-node chunk and the output DMAs out untransposed.
  * all heavy matmuls run in bf16 (1 cycle/row at any free size; halves
    DMA and SBUF traffic).  Accumulation stays fp32 in PSUM.
  * engine balance: ACT does exp + the bvo PSUM->SBUF copies (exp,
    identity and copy share one activation table, so no table
    thrashing); DVE does the f/b bias-add copies (which gate the score
    matmuls -- keeping them off ACT keeps ACT off the critical path)
    and the per-chunk softmax normalization.
"""

import sys

for _p in ("/opt/trn_rl_repo", "/root/.axon_site/_ro/trn_rl_repo"):
    if _p not in sys.path:
        sys.path.append(_p)

import numpy as np

import bass_rust

import concourse.bass as bass
import concourse.mybir as mybir
from concourse.bass_utils import run_bass_kernel_spmd
from concourse.tile import TileContext
from concourse.vector_clock import ScopedClock, VectorClock

F32 = mybir.dt.float32
BF16 = mybir.dt.bfloat16
AL = mybir.AluOpType

# walrus build here rejects >1 semaphore wait per instruction; split the
# excess onto nops (same workaround as the baseline kernel).
_MAX_WAITS = 1

M, B, N, FD, BD, BVD, H = 4096, 8, 512, 128, 128, 6, 64
H2 = H // 2
N_CORES = 8
MW_DEFAULT = 544


class _ChunkedDrainTileContext(TileContext):
    _nop_uid = 0

    def _add_instruction(self, inst):
        si = inst.sync_info
        if (
            si is not None
            and si.on_wait
            and len(si.on_wait) > _MAX_WAITS
            and inst.engine != mybir.EngineType.Unassigned
        ):
            waits = list(si.on_wait)
            excess, keep = waits[:-_MAX_WAITS], waits[-_MAX_WAITS:]
            for i in range(0, len(excess), _MAX_WAITS):
                _ChunkedDrainTileContext._nop_uid += 1
                nop = mybir.InstNoOp(
                    name=f"splitw{_ChunkedDrainTileContext._nop_uid}", ins=[], outs=[]
                )
                nop.engine = inst.engine
                nop.sync_info = bass_rust.SyncInfo(
                    on_wait=excess[i : i + _MAX_WAITS], on_update=[]
                )
                super()._add_instruction(nop)
            inst.sync_info = bass_rust.SyncInfo(on_wait=keep, on_update=si.on_update)
        super()._add_instruction(inst)

    def _drain_and_barrier(self, tick_clock, wait_clock):
        nc = self.nc
        g = tick_clock.global_clock
        nprocs = len(g)
        for i in range(nprocs):
            if g[i] > 0:
                vc = VectorClock([0] * nprocs)
                vc.require_at_least(i, g[i])
                nop_inst = nc.sync.nop(nofuse=True, hint=f"drain_wait_p{i}")
                wait_clock.add_sem_waits(nop_inst.ins, ScopedClock({None: vc}))
        nc.sync.drain()
        nc.all_engine_barrier()
        assert self.sems is not None
        popped = nc._tile_sem_poison_stack.pop()
        assert popped is self._sem_poison
        nc.clear_and_free_semaphores(list(self.sems.allocated().values()))
        nc.all_engine_barrier()


def _mslots(mw):
    """Slot list [(offset, length)] with length<=256 (PSUM bank size)."""
    slots = []
    off = 0
    while off < mw:
        slots.append((off, min(256, mw - off)))
        off += 256
    return slots


def _mchunks(mw):
    """Apply chunks [(offset, length)] with length<=128 (stationary)."""
    chunks = []
    off = 0
    while off < mw:
        chunks.append((off, min(128, mw - off)))
        off += 128
    return chunks


def build_program(reps=1, mw=MW_DEFAULT):
    nc = bass.Bass()

    d_f = nc.declare_dram_parameter("ft", [FD, mw], BF16, isOutput=False)
    d_b = nc.declare_dram_parameter("bp", [BD, N], BF16, isOutput=False)
    d_bv = nc.declare_dram_parameter("bvp", [8, N], BF16, isOutput=False)
    # wbf: WfT 0:64 | WbT 64:128 | wbvo_e 128:194 (rows 0:8) | wbvo_g 194:260
    d_wb = nc.declare_dram_parameter("wbf", [128, 260], BF16, isOutput=False)
    # wsc: col0 bf, col1 bb
    d_ws = nc.declare_dram_parameter("wsc", [64, 2], F32, isOutput=False)
    d_out = nc.declare_dram_parameter("outm", [mw, H], F32, isOutput=True)

    slots = _mslots(mw)
    chunks = _mchunks(mw)

    with _ChunkedDrainTileContext(nc) as tc, nc.allow_low_precision(
        reason="bf16 compute of fp32 data"
    ):
        with (
            tc.tile_pool(name="const", bufs=1) as cp,
            tc.tile_pool(name="io", bufs=2) as iop,
            tc.tile_pool(name="wk", bufs=2) as wkp,
            tc.tile_pool(name="te", bufs=3) as tep,
            tc.tile_pool(name="ps_f", bufs=1, space="PSUM") as psp_f,
            tc.tile_pool(name="ps_v", bufs=1, space="PSUM") as psp_v,
            tc.tile_pool(name="ps_s", bufs=2, space="PSUM") as psp_s,
            tc.tile_pool(name="ps_a", bufs=2, space="PSUM") as psp_a,
        ):
            t_w = cp.tile([128, 260], BF16, tag="wbf")
            nc.sync.dma_start(t_w[:], d_wb[:])
            t_wft = t_w[:, 0:64]
            t_wbt = t_w[:, 64:128]
            t_wbv = {"e": t_w[0:8, 128:194], "g": t_w[0:8, 194:260]}
            t_ws = cp.tile([64, 2], F32, tag="wsc")
            nc.scalar.dma_start(t_ws[:], d_ws[:])
            t_bfc = t_ws[:, 0:1]
            t_bbc = t_ws[:, 1:2]
            t_bvp = cp.tile([8, N], BF16, tag="bvp")
            nc.gpsimd.dma_start(t_bvp[:], d_bv[:])

            for rep in range(reps):
                # ---- loads ----
                t_ft = iop.tile([FD, mw], BF16, tag="f")
                nc.sync.dma_start(t_ft[:], d_f[:])
                t_bt = iop.tile([BD, N], BF16, tag="b")
                nc.scalar.dma_start(t_bt[:], d_b[:])

                # ---- boundary features b = Wb @ bp + bb   (64, 512) ----
                ps_b = psp_f.tile([H, N], F32, tag="feat")
                nc.tensor.matmul(ps_b[:], t_wbt, t_bt[:], start=True, stop=True)
                t_bsb = wkp.tile([H, N], BF16, tag="bsb")
                nc.vector.tensor_scalar_add(t_bsb[:], ps_b[:], t_bbc)

                # ---- node features f = Wf @ fT + bf   (64, mw) bf16 ----
                t_fsb = wkp.tile([H, mw], BF16, tag="fsb")
                for so, sl in slots:
                    ps = psp_f.tile([H, N], F32, tag="feat")
                    nc.tensor.matmul(
                        ps[:, 0:sl], t_wft, t_ft[:, so : so + sl],
                        start=True, stop=True,
                    )
                    nc.vector.tensor_scalar_add(
                        t_fsb[:, so : so + sl], ps[:, 0:sl], t_bfc
                    )

                # ---- bvo_h[n, j] = (Wo_h@Wbv @ bv)[j,n] + fold, col64=1 ----
                t_bvo = {}
                for hx in ("e", "g"):
                    ps_v = psp_v.tile([128, 264], F32, tag="bvo")
                    for k in range(4):
                        nc.tensor.matmul(
                            ps_v[:, 66 * k : 66 * k + 66],
                            t_bvp[:, 128 * k : 128 * k + 128],
                            t_wbv[hx],
                            start=True, stop=True,
                        )
                    tb = wkp.tile([128, 264], BF16, tag=f"bvo{hx}")
                    nc.scalar.activation(
                        tb[:], ps_v[:], mybir.ActivationFunctionType.Identity,
                    )
                    t_bvo[hx] = tb

                # ---- scores -> exp -> apply -> normalize, per slot ----
                te = {}
                ps_apply = {}

                def emit_scores(si_):
                    so, sl = slots[si_]
                    for hi, hx in ((0, "e"), (1, "g")):
                        h0 = hi * H2
                        ps_s = psp_s.tile([128, 1024], F32, tag="s")
                        for k in range(4):
                            nc.tensor.matmul(
                                ps_s[:, sl * k : sl * k + sl],
                                t_bsb[h0 : h0 + H2, 128 * k : 128 * k + 128],
                                t_fsb[h0 : h0 + H2, so : so + sl],
                                start=True, stop=True,
                            )
                        t_e = tep.tile([128, 1024], BF16, tag="te")
                        nc.scalar.activation(
                            t_e[:, 0 : 4 * sl],
                            ps_s[:, 0 : 4 * sl],
                            mybir.ActivationFunctionType.Exp,
                            scale=0.125,
                        )
                        te[(si_, hx)] = t_e

                def emit_apply(si_):
                    so, sl = slots[si_]
                    ps_a = psp_a.tile([128, 264], F32, tag="ap")
                    segs = []
                    for j in range(0, sl, 128):
                        ml = min(128, sl - j)
                        for hx in ("e", "g"):
                            seg = 66 * len(segs)
                            for k in range(4):
                                nc.tensor.matmul(
                                    ps_a[0:ml, seg : seg + 66],
                                    te[(si_, hx)][:, sl * k + j : sl * k + j + ml],
                                    t_bvo[hx][:, 66 * k : 66 * k + 66],
                                    start=(k == 0), stop=(k == 3),
                                )
                            segs.append((j, ml, hx, seg))
                    ps_apply[si_] = (ps_a, segs)

                def emit_norm(si_):
                    so, sl = slots[si_]
                    ps_a, segs = ps_apply[si_]
                    for i in range(0, len(segs), 2):
                        j, ml, _, seg_e = segs[i]
                        _, _, _, seg_g = segs[i + 1]
                        t_r = wkp.tile([128, 2], F32, tag="recip")
                        nc.vector.reciprocal(
                            t_r[0:ml, 0:1], ps_a[0:ml, seg_e + 64 : seg_e + 65]
                        )
                        nc.vector.reciprocal(
                            t_r[0:ml, 1:2], ps_a[0:ml, seg_g + 64 : seg_g + 65]
                        )
                        t_t = wkp.tile([128, H], F32, tag="tmp")
                        nc.vector.tensor_scalar_mul(
                            t_t[0:ml, :], ps_a[0:ml, seg_e : seg_e + 64],
                            t_r[0:ml, 0:1],
                        )
                        t_f_ = iop.tile([128, H], F32, tag="fin")
                        nc.vector.scalar_tensor_tensor(
                            t_f_[0:ml, :],
                            ps_a[0:ml, seg_g : seg_g + 64],
                            t_r[0:ml, 1:2],
                            t_t[0:ml, :],
                            AL.mult,
                            AL.add,
                        )
                        nc.sync.dma_start(
                            d_out[so + j : so + j + ml, :], t_f_[0:ml, :]
                        )

                # interleave so PE stays busy while ACT/DVE trail
                emit_scores(0)
                if len(slots) > 1:
                    emit_scores(1)
                emit_apply(0)
                for si_ in range(2, len(slots)):
                    emit_scores(si_)
                emit_norm(0)
                for si_ in range(1, len(slots)):
                    emit_apply(si_)
                    emit_norm(si_)

    return nc


def plan(batch):
    batch = np.asarray(batch).astype(np.int64)
    bounds = np.searchsorted(batch, np.arange(B + 1))
    sizes = np.diff(bounds)
    mw = max(288, int(-(-sizes.max() // 32) * 32))
    return bounds, mw


def stage_inputs(inputs, bounds, mw):
    f_pre_in = np.asarray(inputs["f_pre_in"], dtype=np.float32)
    b_pre_in = np.asarray(inputs["b_pre_in"], dtype=np.float32)
    bv_in = np.asarray(inputs["bv_in"], dtype=np.float32)
    Wf = np.asarray(inputs["Wf"], dtype=np.float32)
    bf = np.asarray(inputs["bf"], dtype=np.float32)
    Wb = np.asarray(inputs["Wb"], dtype=np.float32)
    bb = np.asarray(inputs["bb"], dtype=np.float32)
    Wbv = np.asarray(inputs["Wbv"], dtype=np.float32)
    bbv = np.asarray(inputs["bbv"], dtype=np.float32)
    Wo = np.asarray(inputs["Wo"], dtype=np.float32)
    bo = np.asarray(inputs["bo"], dtype=np.float32)

    wbf = np.zeros((128, 260), np.float32)
    wbf[:, 0:64] = Wf.T
    wbf[:, 64:128] = Wb.T
    for ci, h0 in ((0, 0), (1, H)):
        wo_h = Wo[:, h0 : h0 + H]                     # (64, 64)
        wv = wo_h @ Wbv                               # (64, 6)
        cv = wo_h @ bbv + 0.5 * bo                    # (64,)
        blk = np.zeros((8, 66), np.float32)
        blk[0:6, 0:64] = wv.T
        blk[6, 0:64] = cv
        blk[6, 64] = 1.0
        wbf[0:8, 128 + 66 * ci : 194 + 66 * ci] = blk
    wsc = np.stack([bf, bb], axis=1)

    in_maps = []
    for g in range(N_CORES):
        s, e = int(bounds[g]), int(bounds[g + 1])
        ft = np.zeros((FD, mw), np.float32)
        ft[:, 0 : e - s] = f_pre_in[s:e].T
        bvp = np.zeros((8, N), np.float32)
        bvp[0:6] = bv_in[g]
        bvp[6] = 1.0
        in_maps.append(
            {
                "ft": ft.astype(np.float32),  # cast below
                "bp": b_pre_in[g],
                "bvp": bvp,
                "wbf": wbf,
                "wsc": wsc,
            }
        )
    # bf16 conversion via jax-free numpy view trick: round-to-nearest-even
    import ml_dtypes

    for m_ in in_maps:
        for k in ("ft", "bp", "bvp", "wbf"):
            m_[k] = np.asarray(m_[k], dtype=ml_dtypes.bfloat16)
    return in_maps


def unstage_output(results, bounds, mw):
    out = np.zeros((M, H), np.float32)
    for g in range(N_CORES):
        s, e = int(bounds[g]), int(bounds[g + 1])
        out[s:e] = results[g]["outm"][0 : e - s]
    return out


_NC_CACHE = {}


def kernel(**inputs):
    assert np.asarray(inputs["f_pre_in"]).shape == (M, FD)
    bounds, mw = plan(inputs["f_pre_batch"])
    in_maps = stage_inputs(inputs, bounds, mw)
    if mw not in _NC_CACHE:
        _NC_CACHE[mw] = build_program(mw=mw)
    nc = _NC_CACHE[mw]
    res = run_bass_kernel_spmd(nc, in_maps, core_ids=list(range(N_CORES)))
    return unstage_output(res.results, bounds, mw)


if __name__ == "__main__":
    rng = np.random.default_rng(0)
    demo = {
        "f_pre_in": rng.standard_normal((M, FD), dtype=np.float32),
        "f_pre_batch": np.sort(rng.integers(0, B, size=M)),
        "b_pre_in": rng.standard_normal((B, BD, N), dtype=np.float32),
        "bv_in": rng.standard_normal((B, BVD, N), dtype=np.float32),
        "Wf": rng.standard_normal((H, FD), dtype=np.float32) * 0.05,
        "bf": rng.standard_normal(H, dtype=np.float32) * 0.05,
        "Wb": rng.standard_normal((H, BD), dtype=np.float32) * 0.05,
        "bb": rng.standard_normal(H, dtype=np.float32) * 0.05,
        "Wbv": rng.standard_normal((H, BVD), dtype=np.float32) * 0.05,
        "bbv": rng.standard_normal(H, dtype=np.float32) * 0.05,
        "Wo": rng.standard_normal((H, 2 * H), dtype=np.float32) * 0.05,
        "bo": rng.standard_normal(H, dtype=np.float32) * 0.05,
    }
    out = kernel(**demo)
    print("kernel output", out.shape, out.dtype, float(np.abs(out).mean()))
